# revision 5
# baseline (speedup 1.0000x reference)
"""Trainium2 distributed kernel: 4-layer attention encoder (B=4, D=1024, H=16, N=1024).

Sharding: sequence-parallel over N across 8 NeuronCores (128 columns each).
All conv1x1 projections + the MLP are purely per-column -> fully local.
Per layer, each core computes its K and V^T shards, AllGathers them
(bf16, one fused collective), and runs attention for its 128 query columns.

Host-side preprocessing (exact, fp32):
  - channel permutation to head-major so each head's 64 channels are contiguous
  - 1/sqrt(DK) folded into Wq/bq
  - bk dropped (constant-per-row shift is softmax invariant)
  - bv folded into the merge bias (softmax rows sum to 1): bm_eff = bm + Wm @ bv
  - BatchNorm (eval) + p1 bias folded to per-channel scale/bias applied in the
    Relu activation: h = relu(s1 * p1_raw + b1)

Compute dtype: bf16 matmul inputs, fp32 PSUM accumulation; the residual
stream stays fp32 end-to-end.
"""

import numpy as np
import ml_dtypes

import concourse.bass as bass
import concourse.mybir as mybir
import concourse.tile as tile
from concourse import bacc
from concourse.bass_utils import run_bass_kernel_spmd

L, D, H, B, N = 4, 1024, 16, 4, 1024
DK = D // H          # 64
R = 8                # cores
NS = N // R          # 128 per-core sequence columns
FB = B * NS          # 512 = free width of one d-tile (batch-concat)
DT = D // 128        # 8 d-tiles
BF = mybir.dt.bfloat16
F32 = mybir.dt.float32
BFNP = ml_dtypes.bfloat16

# head-major channel permutation: perm[h*64+dk] = dk*16+h
PERM = np.array([dk * H + h for h in range(H) for dk in range(DK)])


def _wtile(w_t):
    """(C, M) weight -> (128, C//128 * M) DRAM layout [p, ct*M + m] for lhsT slicing."""
    c, m = w_t.shape
    return np.ascontiguousarray(
        w_t.reshape(c // 128, 128, m).transpose(1, 0, 2).reshape(128, -1)
    ).astype(BFNP)


def _btile(b_vec):
    """(C,) bias -> (128, C//128) [p, ct]."""
    c = b_vec.shape[0]
    return np.ascontiguousarray(b_vec.reshape(c // 128, 128).T).astype(np.float32)


def prepare_host_inputs(inputs):
    """Preprocess full weights once; returns dict of global (shard-independent) arrays."""
    Wq, bq = inputs["Wq"], inputs["bq"]
    Wk = inputs["Wk"]
    Wv, bv = inputs["Wv"], inputs["bv"]
    Wm, bm = inputs["Wm"], inputs["bm"]
    Wp1, bp1 = inputs["Wp1"], inputs["bp1"]
    g, beta = inputs["bn_gamma"], inputs["bn_beta"]
    mu, var = inputs["bn_mean"], inputs["bn_var"]
    Wp2 = inputs["Wp2"]

    out = {k: [] for k in ("wq", "wk", "wv", "wm", "wp1", "wp2", "bq", "bm", "s1", "b1")}
    for l in range(L):
        out["wq"].append(_wtile((Wq[l][PERM] / 8.0).T))
        out["wk"].append(_wtile(Wk[l][PERM].T))
        out["wv"].append(_wtile(Wv[l][PERM].T))
        out["wm"].append(_wtile(Wm[l][:, PERM].T))
        out["wp1"].append(_wtile(Wp1[l].T))
        out["wp2"].append(_wtile(Wp2[l].T))
        out["bq"].append(_btile(bq[l][PERM] / 8.0))
        bm_eff = bm[l] + Wm[l] @ bv[l]
        out["bm"].append(_btile(bm_eff))
        s1 = g[l] / np.sqrt(var[l] + 1e-5)
        b1 = beta[l] + s1 * (bp1[l] - mu[l])
        out["s1"].append(_btile(s1))
        out["b1"].append(_btile(b1))
    res = {k: np.stack(v) for k, v in out.items()}
    # biases: (L, 128, C) -> (128, L*C) so the device DMA is a plain copy
    for k in ("bq", "bm", "s1", "b1"):
        res[k] = np.ascontiguousarray(
            res[k].transpose(1, 0, 2).reshape(128, -1)
        )
    res["ident"] = np.eye(128, dtype=BFNP)
    return res


def shard_x(motion_feats, r):
    """(B, D, N) -> core r's (128, DT*FB) fp32 tile layout [p, ct*512 + b*128 + n]."""
    m = motion_feats[:, :, r * NS : (r + 1) * NS]          # (B, D, NS)
    m = m.reshape(B, DT, 128, NS).transpose(2, 1, 0, 3)    # (p, ct, b, n)
    return np.ascontiguousarray(m.reshape(128, DT * FB)).astype(np.float32)


def unshard_out(res_list):
    """8 x (128, DT*FB) -> (B, D, N)."""
    out = np.empty((B, D, N), dtype=np.float32)
    for r, arr in enumerate(res_list):
        m = arr.reshape(128, DT, B, NS).transpose(2, 1, 0, 3)  # (b, ct, p, n)
        out[:, :, r * NS : (r + 1) * NS] = m.reshape(B, D, NS)
    return out


def build_nc():
    nc = bacc.Bacc(
        "TRN2", target_bir_lowering=False, debug=False, num_devices=R
    )

    x_in = nc.dram_tensor("x_in", [128, DT * FB], F32, kind="ExternalInput")
    wq = nc.dram_tensor("wq", [L, 128, DT * D], BF, kind="ExternalInput")
    wk = nc.dram_tensor("wk", [L, 128, DT * D], BF, kind="ExternalInput")
    wv = nc.dram_tensor("wv", [L, 128, DT * D], BF, kind="ExternalInput")
    wm = nc.dram_tensor("wm", [L, 128, DT * D], BF, kind="ExternalInput")
    wp1 = nc.dram_tensor("wp1", [L, 128, 16 * 2048], BF, kind="ExternalInput")
    wp2 = nc.dram_tensor("wp2", [L, 128, 16 * 1024], BF, kind="ExternalInput")
    bq_d = nc.dram_tensor("bq", [128, L * 8], F32, kind="ExternalInput")
    bm_d = nc.dram_tensor("bm", [128, L * 8], F32, kind="ExternalInput")
    s1_d = nc.dram_tensor("s1", [128, L * 16], F32, kind="ExternalInput")
    b1_d = nc.dram_tensor("b1", [128, L * 16], F32, kind="ExternalInput")
    id_d = nc.dram_tensor("ident", [128, 128], BF, kind="ExternalInput")
    out_e = nc.dram_tensor("out", [128, DT * FB], F32, kind="ExternalOutput")

    ADD = mybir.AluOpType.add
    AF = mybir.ActivationFunctionType

    with tile.TileContext(nc) as tc:
        with (
            tc.tile_pool(name="const", bufs=1) as const,
            tc.tile_pool(name="acts", bufs=1) as acts,
            tc.tile_pool(name="wres", bufs=1) as wres,
            tc.tile_pool(name="wstr", bufs=3) as wstr,
            tc.tile_pool(name="kv", bufs=8) as kvp,
            tc.tile_pool(name="attn_t", bufs=2) as attp,
            tc.tile_pool(name="small", bufs=4) as smallp,
            tc.tile_pool(name="pp", bufs=2, space="PSUM") as ppp,
            tc.tile_pool(name="sc", bufs=1, space="PSUM") as scp,
            tc.tile_pool(name="wtp", bufs=1, space="PSUM") as wtpp,
            tc.tile_pool(name="at", bufs=2, space="PSUM") as atp,
            tc.tile_pool(name="dram", bufs=2, space="DRAM") as dramp,
        ):
            ident = const.tile([128, 128], BF)
            nc.sync.dma_start(ident[:], id_d[:, :])
            bq_sb = const.tile([128, L * 8], F32)
            nc.sync.dma_start(bq_sb[:], bq_d[:, :])
            bm_sb = const.tile([128, L * 8], F32)
            nc.sync.dma_start(bm_sb[:], bm_d[:, :])
            s1_sb = const.tile([128, L * 16], F32)
            nc.sync.dma_start(s1_sb[:], s1_d[:, :])
            b1_sb = const.tile([128, L * 16], F32)
            nc.sync.dma_start(b1_sb[:], b1_d[:, :])

            x_sb = acts.tile([128, DT * FB], F32)
            nc.sync.dma_start(x_sb[:], x_in[:, :])
            x_bf = acts.tile([128, DT * FB], BF)
            q_bf = acts.tile([128, DT * FB], BF)
            attn_bf = acts.tile([128, DT * FB], BF)
            mg_bf = acts.tile([128, DT * FB], BF)
            h1_bf = acts.tile([128, 16 * FB], BF)
            k_sh = acts.tile([128, DT * FB], BF)
            v_sh = acts.tile([128, DT * FB], BF)

            def stream_w(src, l, mt, width, nchunks, tag):
                """lhsT m-tile: all contraction chunks for output tile mt."""
                t = wstr.tile([128, nchunks * 128], BF, tag=tag)
                view = src.ap().rearrange(
                    "l p (ct m) -> l p ct m", ct=nchunks
                )
                nc.sync.dma_start(
                    t[:].rearrange("p (ct m) -> p ct m", ct=nchunks),
                    view[l, :, :, mt * 128 : (mt + 1) * 128],
                )
                return t

            for l in range(L):
                # cast residual stream to bf16 once per layer
                nc.vector.tensor_copy(x_bf[:], x_sb[:])

                # ---- K projection (feeds the collective first) ----
                for mt in range(DT):
                    w_t = stream_w(wk, l, mt, D, DT, "wqk")
                    ps = ppp.tile([128, FB], F32, tag="pp")
                    for ct in range(DT):
                        nc.tensor.matmul(
                            ps[:],
                            w_t[:, ct * 128 : (ct + 1) * 128],
                            x_bf[:, ct * FB : (ct + 1) * FB],
                            start=(ct == 0),
                            stop=(ct == DT - 1),
                        )
                    nc.scalar.copy(k_sh[:, mt * FB : (mt + 1) * FB], ps[:])

                # ---- V^T projection: out[n, d] per batch ----
                wv_sb = wres.tile([128, DT * D], BF, tag="wv")
                nc.sync.dma_start(wv_sb[:], wv[l, :, :])
                for b in range(B):
                    for dh in range(2):
                        ps = ppp.tile([128, FB], F32, tag="pp")
                        for ct in range(DT):
                            nc.tensor.matmul(
                                ps[:],
                                x_bf[:, ct * FB + b * NS : ct * FB + (b + 1) * NS],
                                wv_sb[:, ct * D + dh * 512 : ct * D + (dh + 1) * 512],
                                start=(ct == 0),
                                stop=(ct == DT - 1),
                            )
                        nc.scalar.copy(
                            v_sh[:, b * D + dh * 512 : b * D + (dh + 1) * 512], ps[:]
                        )

                # ---- AllGather K and V^T (bf16, fused buffer) ----
                cc_i = dramp.tile([2 * 128, DT * FB], BF, tag="cci")
                nc.sync.dma_start(cc_i[0:128, :], k_sh[:])
                nc.sync.dma_start(cc_i[128:256, :], v_sh[:])
                cc_o = dramp.tile([R * 2 * 128, DT * FB], BF, tag="cco", addr_space="Shared")
                nc.gpsimd.collective_compute(
                    "AllGather",
                    mybir.AluOpType.bypass,
                    replica_groups=[list(range(R))],
                    ins=[cc_i[:].opt()],
                    outs=[cc_o[:].opt()],
                )

                # ---- Q projection (overlaps the collective) ----
                for mt in range(DT):
                    w_t = stream_w(wq, l, mt, D, DT, "wqk")
                    ps = ppp.tile([128, FB], F32, tag="pp")
                    for ct in range(DT):
                        nc.tensor.matmul(
                            ps[:],
                            w_t[:, ct * 128 : (ct + 1) * 128],
                            x_bf[:, ct * FB : (ct + 1) * FB],
                            start=(ct == 0),
                            stop=(ct == DT - 1),
                        )
                    nc.scalar.activation(
                        q_bf[:, mt * FB : (mt + 1) * FB],
                        ps[:],
                        AF.Identity,
                        bias=bq_sb[:, l * 8 + mt : l * 8 + mt + 1],
                    )

                # prefetch merge weights while attention runs
                wm_sb = wres.tile([128, DT * D], BF, tag="wm")
                nc.sync.dma_start(wm_sb[:], wm[l, :, :])

                # views of gathered K / V^T
                # cc_o rows: r*256 + [0..127] = K shard of rank r, [128..255] = V^T
                ko = cc_o[:].rearrange(
                    "(r s p) (mt b n) -> s r p mt b n", r=R, s=2, p=128, mt=DT, b=B
                )
                vo = cc_o[:].rearrange(
                    "(r s p) (b d) -> s r p b d", r=R, s=2, p=128, b=B
                )

                # ---- attention ----
                for b in range(B):
                    kts = []
                    vts = []
                    for t in range(DT):
                        kt = kvp.tile([128, N], BF, tag="kt")
                        # dst [p, r*128+ms] <- K[d=t*128+p, m=r*128+ms] of batch b
                        nc.sync.dma_start(
                            kt[:].rearrange("p (r m) -> p r m", r=R),
                            ko[0, :, :, t, b, :].rearrange("r p m -> p r m"),
                        )
                        kts.append(kt)
                    for r in range(R):
                        vt = kvp.tile([128, D], BF, tag="vt")
                        # dst [ms, d] <- V^T[m=r*128+ms, d] of batch b
                        nc.sync.dma_start(vt[:], vo[1, r, :, b, :])
                        vts.append(vt)

                    for h in range(H):
                        t, half = h // 2, (h % 2) * 64
                        sc = scp.tile([128, N], F32, tag="sc")
                        lhsT_q = q_bf[
                            half : half + 64, t * FB + b * NS : t * FB + (b + 1) * NS
                        ]
                        for j in range(2):
                            nc.tensor.matmul(
                                sc[:, j * 512 : (j + 1) * 512],
                                lhsT_q,
                                kts[t][half : half + 64, j * 512 : (j + 1) * 512],
                                start=True,
                                stop=True,
                            )
                        w_raw = attp.tile([128, N], BF, tag="wraw")
                        ssum = smallp.tile([128, 1], F32, tag="ssum")
                        nc.scalar.activation(
                            w_raw[:], sc[:], AF.Exp, accum_out=ssum[:]
                        )
                        rinv = smallp.tile([128, 1], F32, tag="rinv")
                        nc.vector.reciprocal(rinv[:], ssum[:])
                        w_bf = attp.tile([128, N], BF, tag="wbf")
                        nc.vector.tensor_scalar_mul(w_bf[:], w_raw[:], rinv[:])

                        wt_ps = wtpp.tile([128, N], BF, tag="wt")
                        for r in range(R):
                            nc.tensor.transpose(
                                wt_ps[:, r * 128 : (r + 1) * 128],
                                w_bf[:, r * 128 : (r + 1) * 128],
                                ident[:],
                            )
                        wt_sb = attp.tile([128, N], BF, tag="wtsb")
                        nc.scalar.copy(wt_sb[:], wt_ps[:])

                        at = atp.tile([64, 128], F32, tag="at")
                        for r in range(R):
                            nc.tensor.matmul(
                                at[:],
                                vts[r][:, h * 64 : (h + 1) * 64],
                                wt_sb[:, r * 128 : (r + 1) * 128],
                                start=(r == 0),
                                stop=(r == R - 1),
                            )
                        nc.scalar.copy(
                            attn_bf[
                                half : half + 64,
                                t * FB + b * NS : t * FB + (b + 1) * NS,
                            ],
                            at[:],
                        )

                # ---- merge ----
                for mt in range(DT):
                    ps = ppp.tile([128, FB], F32, tag="pp")
                    for ct in range(DT):
                        nc.tensor.matmul(
                            ps[:],
                            wm_sb[:, ct * D + mt * 128 : ct * D + (mt + 1) * 128],
                            attn_bf[:, ct * FB : (ct + 1) * FB],
                            start=(ct == 0),
                            stop=(ct == DT - 1),
                        )
                    nc.scalar.activation(
                        mg_bf[:, mt * FB : (mt + 1) * FB],
                        ps[:],
                        AF.Identity,
                        bias=bm_sb[:, l * 8 + mt : l * 8 + mt + 1],
                    )

                # ---- p1 + BN + relu (contraction: 8 merged chunks + 8 x chunks) ----
                for mt in range(16):
                    w_t = stream_w(wp1, l, mt, 2048, 16, "wp1")
                    ps = ppp.tile([128, FB], F32, tag="pp")
                    for ct in range(16):
                        rhs = (
                            mg_bf[:, ct * FB : (ct + 1) * FB]
                            if ct < 8
                            else x_bf[:, (ct - 8) * FB : (ct - 7) * FB]
                        )
                        nc.tensor.matmul(
                            ps[:],
                            w_t[:, ct * 128 : (ct + 1) * 128],
                            rhs,
                            start=(ct == 0),
                            stop=(ct == 15),
                        )
                    nc.scalar.activation(
                        h1_bf[:, mt * FB : (mt + 1) * FB],
                        ps[:],
                        AF.Relu,
                        bias=b1_sb[:, l * 16 + mt : l * 16 + mt + 1],
                        scale=s1_sb[:, l * 16 + mt : l * 16 + mt + 1],
                    )

                # ---- p2 + residual ----
                for ot in range(DT):
                    w_t = stream_w(wp2, l, ot, 1024, 16, "wp2")
                    ps = ppp.tile([128, FB], F32, tag="pp")
                    for ct in range(16):
                        nc.tensor.matmul(
                            ps[:],
                            w_t[:, ct * 128 : (ct + 1) * 128],
                            h1_bf[:, ct * FB : (ct + 1) * FB],
                            start=(ct == 0),
                            stop=(ct == 15),
                        )
                    nc.vector.tensor_tensor(
                        x_sb[:, ot * FB : (ot + 1) * FB],
                        x_sb[:, ot * FB : (ot + 1) * FB],
                        ps[:],
                        op=ADD,
                    )

            nc.sync.dma_start(out_e[:, :], x_sb[:])

    nc.finalize()
    return nc


_CACHED = {}


def kernel(**inputs):
    inputs = {k: np.asarray(v) for k, v in inputs.items()}
    host = prepare_host_inputs(inputs)

    if "nc" not in _CACHED:
        _CACHED["nc"] = build_nc()
    nc = _CACHED["nc"]

    in_maps = []
    for r in range(R):
        m = {
            "x_in": shard_x(inputs["motion_feats"], r),
            "ident": host["ident"],
            "wq": host["wq"], "wk": host["wk"], "wv": host["wv"], "wm": host["wm"],
            "wp1": host["wp1"], "wp2": host["wp2"],
            "bq": host["bq"], "bm": host["bm"], "s1": host["s1"], "b1": host["b1"],
        }
        in_maps.append(m)

    res = run_bass_kernel_spmd(nc, in_maps, core_ids=list(range(R)))
    return unshard_out([res.results[r]["out"] for r in range(R)])


# revision 8
# speedup vs baseline: 1.5131x; 1.5131x over previous
"""Trainium2 distributed kernel: 4-layer attention encoder (B=4, D=1024, H=16, N=1024).

Sharding: (batch, sequence-half) across 8 NeuronCores — core r owns batch
b = r//2 and sequence half r%2 (512 columns). All conv1x1 projections and
the MLP are per-column -> fully local. Per layer each core computes its
K / V^T shard and AllGathers it with its batch peer only (2-rank groups),
then runs attention for its 512 query columns of its batch.

Host-side preprocessing (exact, fp32):
  - channel permutation to head-major so each head's 64 channels are contiguous
  - 1/sqrt(DK) folded into Wq/bq
  - bk dropped (constant-per-row shift is softmax invariant)
  - bv folded into the merge bias (softmax rows sum to 1): bm_eff = bm + Wm @ bv
  - BatchNorm (eval) + p1 bias folded to per-channel scale/bias applied in the
    Relu activation: h = relu(s1 * p1_raw + b1)
  - streamed lhsT weights packed per output-tile so every weight DMA is one
    contiguous block

Compute dtype: bf16 matmul inputs, fp32 PSUM accumulation; the residual
stream stays fp32 end-to-end.
"""

import numpy as np
import ml_dtypes

import concourse.bass as bass
import concourse.mybir as mybir
import concourse.tile as tile
from concourse import bacc
from concourse.bass_utils import run_bass_kernel_spmd

L, D, H, B, N = 4, 1024, 16, 4, 1024
DK = D // H          # 64
R = 8                # cores
NS = N // 2          # 512 per-core sequence columns (one batch, half sequence)
DT = D // 128        # 8 d-tiles
NT = NS // 128       # 4 n-tiles per core
BF = mybir.dt.bfloat16
F32 = mybir.dt.float32
BFNP = ml_dtypes.bfloat16

# head-major channel permutation: perm[h*64+dk] = dk*16+h
PERM = np.array([dk * H + h for h in range(H) for dk in range(DK)])


def _wtile_stream(w_t):
    """(C, M) weight -> (M//128, 128, C//128*128): arr[mt, p, ct*128+mo] =
    w_t[ct*128+p, mt*128+mo]. Each [mt] block is one contiguous lhsT tile."""
    c, m = w_t.shape
    a = w_t.reshape(c // 128, 128, m // 128, 128)      # (ct, p, mt, mo)
    a = a.transpose(2, 1, 0, 3)                        # (mt, p, ct, mo)
    return np.ascontiguousarray(a.reshape(m // 128, 128, -1)).astype(BFNP)


def _wtile_res(w_t):
    """(C, M) weight -> (128, C//128*M) [p, ct*M + m] for resident rhs/lhsT use."""
    c, m = w_t.shape
    return np.ascontiguousarray(
        w_t.reshape(c // 128, 128, m).transpose(1, 0, 2).reshape(128, -1)
    ).astype(BFNP)


def _btile(b_vec):
    """(C,) bias -> (128, C//128) [p, ct]."""
    c = b_vec.shape[0]
    return np.ascontiguousarray(b_vec.reshape(c // 128, 128).T).astype(np.float32)


def prepare_host_inputs(inputs):
    """Preprocess full weights once; returns dict of shard-independent arrays."""
    Wq, bq = inputs["Wq"], inputs["bq"]
    Wk = inputs["Wk"]
    Wv, bv = inputs["Wv"], inputs["bv"]
    Wm, bm = inputs["Wm"], inputs["bm"]
    Wp1, bp1 = inputs["Wp1"], inputs["bp1"]
    g, beta = inputs["bn_gamma"], inputs["bn_beta"]
    mu, var = inputs["bn_mean"], inputs["bn_var"]
    Wp2 = inputs["Wp2"]

    out = {k: [] for k in ("wq", "wk", "wv", "wm", "wp1", "wp2", "bq", "bm", "s1", "b1")}
    for l in range(L):
        out["wq"].append(_wtile_stream((Wq[l][PERM] / 8.0).T))
        out["wk"].append(_wtile_stream(Wk[l][PERM].T))
        out["wv"].append(_wtile_res(Wv[l][PERM].T))
        out["wm"].append(_wtile_res(Wm[l][:, PERM].T))
        out["wp1"].append(_wtile_stream(Wp1[l].T))
        out["wp2"].append(_wtile_stream(Wp2[l].T))
        out["bq"].append(_btile(bq[l][PERM] / 8.0))
        bm_eff = bm[l] + Wm[l] @ bv[l]
        out["bm"].append(_btile(bm_eff))
        s1 = g[l] / np.sqrt(var[l] + 1e-5)
        b1 = beta[l] + s1 * (bp1[l] - mu[l])
        out["s1"].append(_btile(s1))
        out["b1"].append(_btile(b1))
    res = {k: np.stack(v) for k, v in out.items()}
    # biases: (L, 128, C) -> (128, L*C) so the device DMA is a plain copy
    for k in ("bq", "bm", "s1", "b1"):
        res[k] = np.ascontiguousarray(res[k].transpose(1, 0, 2).reshape(128, -1))
    res["ident"] = np.eye(128, dtype=BFNP)
    return res


def shard_x(motion_feats, r):
    """(B, D, N) -> core r's (128, DT*NS) fp32 tile layout [p, ct*NS + n]."""
    b, half = r // 2, r % 2
    m = motion_feats[b, :, half * NS : (half + 1) * NS]    # (D, NS)
    m = m.reshape(DT, 128, NS).transpose(1, 0, 2)          # (p, ct, n)
    return np.ascontiguousarray(m.reshape(128, DT * NS)).astype(np.float32)


def unshard_out(res_list):
    """8 x (128, DT*NS) -> (B, D, N)."""
    out = np.empty((B, D, N), dtype=np.float32)
    for r, arr in enumerate(res_list):
        b, half = r // 2, r % 2
        m = arr.reshape(128, DT, NS).transpose(1, 0, 2)    # (ct, p, n)
        out[b, :, half * NS : (half + 1) * NS] = m.reshape(D, NS)
    return out


def build_nc():
    nc = bacc.Bacc("TRN2", target_bir_lowering=False, debug=False, num_devices=R)

    x_in = nc.dram_tensor("x_in", [128, DT * NS], F32, kind="ExternalInput")
    wq = nc.dram_tensor("wq", [L, DT, 128, D], BF, kind="ExternalInput")
    wk = nc.dram_tensor("wk", [L, DT, 128, D], BF, kind="ExternalInput")
    wv = nc.dram_tensor("wv", [L, 128, DT * D], BF, kind="ExternalInput")
    wm = nc.dram_tensor("wm", [L, 128, DT * D], BF, kind="ExternalInput")
    wp1 = nc.dram_tensor("wp1", [L, 16, 128, 2048], BF, kind="ExternalInput")
    wp2 = nc.dram_tensor("wp2", [L, DT, 128, 2048], BF, kind="ExternalInput")
    bq_d = nc.dram_tensor("bq", [128, L * 8], F32, kind="ExternalInput")
    bm_d = nc.dram_tensor("bm", [128, L * 8], F32, kind="ExternalInput")
    s1_d = nc.dram_tensor("s1", [128, L * 16], F32, kind="ExternalInput")
    b1_d = nc.dram_tensor("b1", [128, L * 16], F32, kind="ExternalInput")
    id_d = nc.dram_tensor("ident", [128, 128], BF, kind="ExternalInput")
    out_e = nc.dram_tensor("out", [128, DT * NS], F32, kind="ExternalOutput")

    ADD = mybir.AluOpType.add
    AF = mybir.ActivationFunctionType
    GROUPS = [[0, 1], [2, 3], [4, 5], [6, 7]]

    with tile.TileContext(nc) as tc:
        with (
            tc.tile_pool(name="const", bufs=1) as const,
            tc.tile_pool(name="acts", bufs=1) as acts,
            tc.tile_pool(name="wres", bufs=1) as wres,
            tc.tile_pool(name="wstr", bufs=3) as wstr,
            tc.tile_pool(name="kv", bufs=8) as kvp,
            tc.tile_pool(name="attn_t", bufs=2) as attp,
            tc.tile_pool(name="small", bufs=6) as smallp,
            tc.tile_pool(name="pp", bufs=2, space="PSUM") as ppp,
            tc.tile_pool(name="sc", bufs=2, space="PSUM") as scp,
            tc.tile_pool(name="wtp", bufs=1, space="PSUM") as wtpp,
            tc.tile_pool(name="at", bufs=1, space="PSUM") as atp,
            tc.tile_pool(name="dram", bufs=2, space="DRAM") as dramp,
        ):
            ident = const.tile([128, 128], BF)
            nc.sync.dma_start(ident[:], id_d[:, :])
            bq_sb = const.tile([128, L * 8], F32)
            nc.sync.dma_start(bq_sb[:], bq_d[:, :])
            bm_sb = const.tile([128, L * 8], F32)
            nc.sync.dma_start(bm_sb[:], bm_d[:, :])
            s1_sb = const.tile([128, L * 16], F32)
            nc.sync.dma_start(s1_sb[:], s1_d[:, :])
            b1_sb = const.tile([128, L * 16], F32)
            nc.sync.dma_start(b1_sb[:], b1_d[:, :])

            x_sb = acts.tile([128, DT * NS], F32)
            nc.sync.dma_start(x_sb[:], x_in[:, :])
            x_bf = acts.tile([128, DT * NS], BF)
            q_bf = acts.tile([128, DT * NS], BF)
            attn_bf = acts.tile([128, DT * NS], BF)
            mg_bf = acts.tile([128, DT * NS], BF)
            h1_bf = acts.tile([128, 16 * NS], BF)
            k_sh = acts.tile([128, DT * NS], BF)
            v_sh = acts.tile([128, NT * D], BF)

            def stream_w(src, l, mt, tag):
                """One contiguous lhsT m-tile: all contraction chunks for mt."""
                t = wstr.tile([128, src.shape[3]], BF, tag=tag)
                nc.sync.dma_start(t[:], src[l, mt, :, :])
                return t

            for l in range(L):
                # cast residual stream to bf16 once per layer
                nc.vector.tensor_copy(x_bf[:], x_sb[:])

                # ---- K projection (feeds the collective first) ----
                for mt in range(DT):
                    w_t = stream_w(wk, l, mt, "wqk")
                    ps = ppp.tile([128, NS], F32, tag="pp")
                    for ct in range(DT):
                        nc.tensor.matmul(
                            ps[:],
                            w_t[:, ct * 128 : (ct + 1) * 128],
                            x_bf[:, ct * NS : (ct + 1) * NS],
                            start=(ct == 0),
                            stop=(ct == DT - 1),
                        )
                    nc.vector.tensor_copy(k_sh[:, mt * NS : (mt + 1) * NS], ps[:])

                # ---- V^T projection: out[n, d], n-tiles of 128 ----
                wv_sb = wres.tile([128, DT * D], BF, tag="wv")
                nc.sync.dma_start(wv_sb[:], wv[l, :, :])
                for nt in range(NT):
                    for dh in range(2):
                        ps = ppp.tile([128, NS], F32, tag="pp")
                        for ct in range(DT):
                            nc.tensor.matmul(
                                ps[:],
                                x_bf[:, ct * NS + nt * 128 : ct * NS + (nt + 1) * 128],
                                wv_sb[:, ct * D + dh * 512 : ct * D + (dh + 1) * 512],
                                start=(ct == 0),
                                stop=(ct == DT - 1),
                            )
                        nc.vector.tensor_copy(
                            v_sh[:, nt * D + dh * 512 : nt * D + (dh + 1) * 512], ps[:]
                        )

                # ---- AllGather K and V^T with the batch peer (2-rank) ----
                cc_i = dramp.tile([2 * 128, DT * NS], BF, tag="cci")
                nc.sync.dma_start(cc_i[0:128, :], k_sh[:])
                nc.sync.dma_start(cc_i[128:256, :], v_sh[:])
                cc_o = dramp.tile([2 * 2 * 128, DT * NS], BF, tag="cco")
                nc.gpsimd.collective_compute(
                    "AllGather",
                    mybir.AluOpType.bypass,
                    replica_groups=GROUPS,
                    ins=[cc_i[:].opt()],
                    outs=[cc_o[:].opt()],
                )

                # ---- Q projection (overlaps the collective) ----
                for mt in range(DT):
                    w_t = stream_w(wq, l, mt, "wqk")
                    ps = ppp.tile([128, NS], F32, tag="pp")
                    for ct in range(DT):
                        nc.tensor.matmul(
                            ps[:],
                            w_t[:, ct * 128 : (ct + 1) * 128],
                            x_bf[:, ct * NS : (ct + 1) * NS],
                            start=(ct == 0),
                            stop=(ct == DT - 1),
                        )
                    nc.vector.tensor_scalar_add(
                        q_bf[:, mt * NS : (mt + 1) * NS],
                        ps[:],
                        bq_sb[:, l * 8 + mt : l * 8 + mt + 1],
                    )

                # prefetch merge weights while attention runs
                wm_sb = wres.tile([128, DT * D], BF, tag="wm")
                nc.sync.dma_start(wm_sb[:], wm[l, :, :])

                # gathered views: cc_o rows r2*256 + s*128 + p
                #   s=0: K shard of rank r2 [p, mt*NS + ns] (d = mt*128+p, n = r2*NS+ns)
                #   s=1: V^T shard [p, nt*D + d] (m = r2*NS + nt*128 + p)
                ko = cc_o[:].rearrange(
                    "(r s p) (mt ns) -> s r p mt ns", r=2, s=2, p=128, mt=DT
                )
                vo = cc_o[:].rearrange(
                    "(r s p) (nt d) -> s r p nt d", r=2, s=2, p=128, nt=NT
                )

                # K tiles: kt[t][p, m] (d = t*128+p, m = 0..1023)
                kts = []
                for t in range(DT):
                    kt = kvp.tile([128, N], BF, tag="kt")
                    nc.sync.dma_start(
                        kt[:].rearrange("p (r ns) -> p r ns", r=2),
                        ko[0, :, :, t, :].rearrange("r p ns -> p r ns"),
                    )
                    kts.append(kt)
                # V^T tiles: vt[mc][p, d] (m = mc*128+p)
                vts = []
                for mc in range(DT):
                    vt = kvp.tile([128, D], BF, tag="vt")
                    nc.sync.dma_start(vt[:], vo[1, mc // NT, :, mc % NT, :])
                    vts.append(vt)

                # ---- attention: 16 heads x 4 query n-tiles ----
                for h in range(H):
                    t, half = h // 2, (h % 2) * 64
                    for nt in range(NT):
                        sc = scp.tile([128, N], F32, tag="sc")
                        lhsT_q = q_bf[
                            half : half + 64, t * NS + nt * 128 : t * NS + (nt + 1) * 128
                        ]
                        for j in range(2):
                            nc.tensor.matmul(
                                sc[:, j * 512 : (j + 1) * 512],
                                lhsT_q,
                                kts[t][half : half + 64, j * 512 : (j + 1) * 512],
                                start=True,
                                stop=True,
                            )
                        w_raw = attp.tile([128, N], BF, tag="wraw")
                        ssum = smallp.tile([128, 1], F32, tag="ssum")
                        nc.scalar.activation(w_raw[:], sc[:], AF.Exp, accum_out=ssum[:])
                        rinv = smallp.tile([128, 1], F32, tag="rinv")
                        nc.vector.reciprocal(rinv[:], ssum[:])
                        w_bf = attp.tile([128, N], BF, tag="wbf")
                        nc.vector.tensor_scalar_mul(w_bf[:], w_raw[:], rinv[:])

                        wt_ps = wtpp.tile([128, N], BF, tag="wt")
                        for r in range(DT):
                            nc.tensor.transpose(
                                wt_ps[:, r * 128 : (r + 1) * 128],
                                w_bf[:, r * 128 : (r + 1) * 128],
                                ident[:],
                            )
                        wt_sb = attp.tile([128, N], BF, tag="wtsb")
                        # alternate the big psum->sbuf copy between ACT and DVE
                        if (h * NT + nt) % 2 == 0:
                            nc.scalar.copy(wt_sb[:], wt_ps[:])
                        else:
                            nc.vector.tensor_copy(wt_sb[:], wt_ps[:])

                        at = atp.tile([64, 128], F32, tag="at")
                        for r in range(DT):
                            nc.tensor.matmul(
                                at[:],
                                vts[r][:, h * 64 : (h + 1) * 64],
                                wt_sb[:, r * 128 : (r + 1) * 128],
                                start=(r == 0),
                                stop=(r == DT - 1),
                            )
                        nc.vector.tensor_copy(
                            attn_bf[
                                half : half + 64,
                                t * NS + nt * 128 : t * NS + (nt + 1) * 128,
                            ],
                            at[:],
                        )

                # ---- merge ----
                for mt in range(DT):
                    ps = ppp.tile([128, NS], F32, tag="pp")
                    for ct in range(DT):
                        nc.tensor.matmul(
                            ps[:],
                            wm_sb[:, ct * D + mt * 128 : ct * D + (mt + 1) * 128],
                            attn_bf[:, ct * NS : (ct + 1) * NS],
                            start=(ct == 0),
                            stop=(ct == DT - 1),
                        )
                    nc.vector.tensor_scalar_add(
                        mg_bf[:, mt * NS : (mt + 1) * NS],
                        ps[:],
                        bm_sb[:, l * 8 + mt : l * 8 + mt + 1],
                    )

                # ---- p1 + BN + relu (contraction: 8 merged chunks + 8 x chunks) ----
                for mt in range(16):
                    w_t = stream_w(wp1, l, mt, "wp1")
                    ps = ppp.tile([128, NS], F32, tag="pp")
                    for ct in range(16):
                        rhs = (
                            mg_bf[:, ct * NS : (ct + 1) * NS]
                            if ct < 8
                            else x_bf[:, (ct - 8) * NS : (ct - 7) * NS]
                        )
                        nc.tensor.matmul(
                            ps[:],
                            w_t[:, ct * 128 : (ct + 1) * 128],
                            rhs,
                            start=(ct == 0),
                            stop=(ct == 15),
                        )
                    nc.scalar.activation(
                        h1_bf[:, mt * NS : (mt + 1) * NS],
                        ps[:],
                        AF.Relu,
                        bias=b1_sb[:, l * 16 + mt : l * 16 + mt + 1],
                        scale=s1_sb[:, l * 16 + mt : l * 16 + mt + 1],
                    )

                # ---- p2 + residual ----
                for ot in range(DT):
                    w_t = stream_w(wp2, l, ot, "wp2")
                    ps = ppp.tile([128, NS], F32, tag="pp")
                    for ct in range(16):
                        nc.tensor.matmul(
                            ps[:],
                            w_t[:, ct * 128 : (ct + 1) * 128],
                            h1_bf[:, ct * NS : (ct + 1) * NS],
                            start=(ct == 0),
                            stop=(ct == 15),
                        )
                    nc.vector.tensor_tensor(
                        x_sb[:, ot * NS : (ot + 1) * NS],
                        x_sb[:, ot * NS : (ot + 1) * NS],
                        ps[:],
                        op=ADD,
                    )

            nc.sync.dma_start(out_e[:, :], x_sb[:])

    nc.finalize()
    return nc


_CACHED = {}


def kernel(**inputs):
    inputs = {k: np.asarray(v) for k, v in inputs.items()}
    host = prepare_host_inputs(inputs)

    if "nc" not in _CACHED:
        _CACHED["nc"] = build_nc()
    nc = _CACHED["nc"]

    in_maps = []
    for r in range(R):
        m = {
            "x_in": shard_x(inputs["motion_feats"], r),
            "ident": host["ident"],
            "wq": host["wq"], "wk": host["wk"], "wv": host["wv"], "wm": host["wm"],
            "wp1": host["wp1"], "wp2": host["wp2"],
            "bq": host["bq"], "bm": host["bm"], "s1": host["s1"], "b1": host["b1"],
        }
        in_maps.append(m)

    res = run_bass_kernel_spmd(nc, in_maps, core_ids=list(range(R)))
    return unshard_out([res.results[r]["out"] for r in range(R)])
